# revision 1
# baseline (speedup 1.0000x reference)
"""Neural CDE Trainium2 kernel.

Strategy: pure data parallelism over batch B=128 -> 8 cores x 16 rows.
Per core, the T-1=1023-step RK4 scan runs as a fully unrolled sequential
chain. Layout: activations [feature_on_partition, batch_on_free].

Math notes:
  - softplus = Ln(Exp(z)+1) using the natural_log_exp_and_others ACT table
    (the only table covering every transcendental used in the loop: Exp,
    Ln, Relu, Identity). Layer biases ride the ACT bias slot ([P,1] AP).
  - tanh(v) = 1 - 2/(1+exp(2v)): Exp on ACT, min/+1 dual-op tensor_scalar,
    reciprocal_approx_fast on DVE.
  - einsum('bhd,bd->bh', tanh(V), dX) with tanh expanded:
        k*a = a*S - 2a * G.T @ (r * Z)
    S[b] = sum_d dX[b,d] (all-alpha [8,64] matmul), Z[p,b] = dX[b, p//16]
    (selector matmul), G one-hot selectors with -2a baked in. fw2 rows are
    permuted so chunk c / partition p hold (h = 16c + p%16, d = p//16).
  - fb2 enters PSUM first via a rank-4 constant matmul (has_written rule).
  - RK4 combination tracked with affine_then_add ops off the chain.

Sync-wait constraint: this walrus build allows a single on_wait per
Matmult, so ALL constants ship in ONE packed DRAM tensor (one DMA queue =
one semaphore) and the per-step dX slice is staged through a DVE copy so
matmuls only ever wait on one producer engine.
"""

import numpy as np

B, T, D, H, W = 128, 1024, 8, 64, 128
NCORES = 8
BS = B // NCORES          # 16 batch rows per core
NSTEPS_FULL = T - 1       # 1023

_CJ = (1.0 / 3.0, 2.0 / 3.0, 1.0 / 3.0, 1.0)  # u_j / alpha_j for y' accum
_SROW = (0, 1, 1, 2)                   # dX variant per stage
_AVARIANT = (0, 0, 1, 2)               # alpha variant {0.5, 1.0, 1/6}
_AVALS = (0.5, 1.0, 1.0 / 6.0)

# wconst free-dim layout: name -> (partitions, free_offset, free_len)
_L = {}
_off = 0
for _name, _p, _f in [
    ("fw0p", H, W), ("fw1p", W, W), ("fw2p", W, 512),
    ("gneg", 128, 3 * 4 * H), ("ebc", D, 128), ("onesa", D, 3 * H),
    ("b3l", 4, 128), ("b3r", 4, 4 * BS),
    ("iw0p", D, W), ("iw1p", W, W), ("iw2p", W, H),
    ("x0T", D, BS), ("lwT", H, 1),
    ("ib0", W, 1), ("ib1", W, 1), ("ib2", H, 1),
    ("fb0", W, 1), ("fb1", W, 1), ("lbneg", 1, 1),
]:
    _L[_name] = (_p, _off, _f)
    _off += _f
WCONST_F = _off


def _hd_orig(c, p):
    h = 16 * c + (p % 16)
    d = p // 16
    return h * D + d


def build_bass(nsteps):
    import concourse.bass as bass
    import concourse.bacc as bacc
    import concourse.mybir as mybir
    from concourse import tile

    f32 = mybir.dt.float32
    AF = mybir.ActivationFunctionType
    ALU = mybir.AluOpType

    # Bacc (not Bass): its compile() runs move_matmul_waits_to_ldweights +
    # generate_event_semaphores, which legalize multi-wait instructions for
    # walrus (1 on_wait per instruction on TRN2).
    nc = bacc.Bacc(None)

    wc_d = nc.declare_dram_parameter("wconst", [128, WCONST_F], f32, isOutput=False)
    dxt_d = [
        nc.declare_dram_parameter(f"dxt{s}", [D, nsteps * BS], f32, isOutput=False)
        for s in range(3)
    ]
    out_d = nc.declare_dram_parameter("out", [1, BS], f32, isOutput=True)

    with tile.TileContext(nc) as tc:
        with (
            tc.tile_pool(name="const", bufs=1) as cpool,
            tc.tile_pool(name="ybase", bufs=1) as ypool,
            tc.tile_pool(name="acc", bufs=1) as apool,
            tc.tile_pool(name="ycur", bufs=2) as ycpool,
            tc.tile_pool(name="work16", bufs=2) as w16,
            tc.tile_pool(name="work64", bufs=2) as w64,
            tc.tile_pool(name="ps_zb", bufs=1, space="PSUM") as ps_zb,
            tc.tile_pool(name="ps_korr", bufs=1, space="PSUM") as ps_korr,
            tc.tile_pool(name="ps_p1", bufs=2, space="PSUM") as ps_p1,
            tc.tile_pool(name="ps_p2", bufs=1, space="PSUM") as ps_p2,
            tc.tile_pool(name="ps_u", bufs=1, space="PSUM") as ps_u,
            tc.tile_pool(name="ps_p3", bufs=1, space="PSUM") as ps_p3,
            tc.tile_pool(name="ps_kneg", bufs=1, space="PSUM") as ps_kneg,
        ):
            wc = cpool.tile([128, WCONST_F], f32, tag="wconst")
            nc.sync.dma_start(wc[:], wc_d[:])
            dxt = []
            for s in range(3):
                dt_ = cpool.tile([D, nsteps * BS], f32, tag=f"dxt{s}")
                nc.sync.dma_start(dt_[:], dxt_d[s][:])
                dxt.append(dt_)

            def C(name):
                p, o, f = _L[name]
                return wc[0:p, o : o + f]

            # Warm each non-PE engine's vector clock on the const DMAs so
            # later ops never carry a DMA wait alongside an engine wait
            # (single on_wait slot per instruction in this walrus build).
            warm = w16.tile([1, 4], f32, tag="warm")
            nc.scalar.activation(warm[0:1, 0:1], wc[0:1, 0:1], AF.Copy)
            nc.vector.tensor_copy(warm[0:1, 1:2], wc[0:1, 0:1])
            for s in range(3):
                nc.vector.tensor_copy(warm[0:1, 1:2], dxt[s][0:1, 0:1])

            # ---- y0 = init_mlp(x0) ----
            y = ypool.tile([H, BS], f32, tag="y")
            A = apool.tile([H, BS], f32, tag="A")

            pi = ps_p1.tile([W, BS], f32, tag="p1")
            nc.tensor.matmul(pi[:], C("iw0p"), C("x0T"), start=True, stop=True)
            h1 = w16.tile([W, BS], f32, tag="s")
            nc.scalar.activation(h1[:], pi[:], AF.Relu, bias=C("ib0"))
            pi2 = ps_p2.tile([W, BS], f32, tag="p2")
            nc.tensor.matmul(pi2[:], C("iw1p"), h1[:], start=True, stop=True)
            h2 = w16.tile([W, BS], f32, tag="s")
            nc.scalar.activation(h2[:], pi2[:], AF.Relu, bias=C("ib1"))
            pk = ps_kneg.tile([H, BS], f32, tag="kneg")
            nc.tensor.matmul(pk[:], C("iw2p"), h2[:], start=True, stop=True)
            nc.scalar.activation(y[:], pk[:], AF.Identity, bias=C("ib2"))

            # ---- the scan ----
            for t in range(nsteps):
                ycur = y
                for j in range(4):
                    s = _SROW[j]
                    av = _AVARIANT[j]
                    cj = _CJ[j]
                    tb = t * BS

                    # off-chain: stage dX slice via DVE, then Z / korr mms
                    dxs = w16.tile([D, BS], f32, tag="dxs")
                    nc.vector.tensor_copy(dxs[:], dxt[s][:, tb : tb + BS])

                    zb_ps = ps_zb.tile([128, BS], f32, tag="zb")
                    nc.tensor.matmul(zb_ps[:], C("ebc"), dxs[:], start=True, stop=True)
                    zb = w16.tile([128, BS], f32, tag="zb_sb")
                    nc.vector.tensor_copy(zb[:], zb_ps[:])

                    korr = ps_korr.tile([H, BS], f32, tag="korr")
                    oa = C("onesa")
                    nc.tensor.matmul(
                        korr[:], oa[:, av * H : (av + 1) * H], dxs[:],
                        start=True, stop=True,
                    )

                    # chain: MLP layer 1
                    p1 = ps_p1.tile([W, BS], f32, tag="p1")
                    nc.tensor.matmul(p1[:], C("fw0p"), ycur[:], start=True, stop=True)
                    u1 = ps_u.tile([W, BS], f32, tag="u")
                    nc.scalar.activation(u1[:], p1[:], AF.Exp, bias=C("fb0"))
                    s1 = w16.tile([W, BS], f32, tag="s")
                    nc.scalar.activation(s1[:], u1[:], AF.Ln, bias=1.0)

                    # chain: MLP layer 2
                    p2 = ps_p2.tile([W, BS], f32, tag="p2")
                    nc.tensor.matmul(p2[:], C("fw1p"), s1[:], start=True, stop=True)
                    u2 = ps_u.tile([W, BS], f32, tag="u")
                    nc.scalar.activation(u2[:], p2[:], AF.Exp, bias=C("fb1"))
                    s2 = w16.tile([W, BS], f32, tag="s")
                    nc.scalar.activation(s2[:], u2[:], AF.Ln, bias=1.0)

                    # chain: MLP layer 3 (4 chunks) + fb2 rank-4 bias mm
                    p3 = ps_p3.tile([128, 4 * BS], f32, tag="p3")
                    nc.tensor.matmul(p3[:], C("b3l"), C("b3r"), start=True, stop=False)
                    fw2p = C("fw2p")
                    for c in range(4):
                        nc.tensor.matmul(
                            p3[:, c * BS : (c + 1) * BS],
                            fw2p[:, c * 128 : (c + 1) * 128],
                            s2[:],
                            start=False, stop=(c == 3),
                        )

                    # chain: tanh pieces
                    texp = w64.tile([128, 4 * BS], f32, tag="texp")
                    nc.scalar.activation(texp[:], p3[:], AF.Exp, scale=2.0)
                    den = w64.tile([128, 4 * BS], f32, tag="den")
                    nc.vector.tensor_scalar(
                        den[:], texp[:], 1.0e30, 1.0, ALU.min, ALU.add
                    )
                    r = w64.tile([128, 4 * BS], f32, tag="r")
                    nc.vector.reciprocal_approx_fast(r[:], den[:])

                    # chain: rZ = r * Z  (Z broadcast along the 4 chunks)
                    rZ = w64.tile([128, 4, BS], f32, tag="rZ")
                    zb_b = zb[:, :]
                    zb_b = bass.AP(
                        zb_b.tensor, zb_b.offset,
                        [zb_b.ap[0], [0, 4], zb_b.ap[1]],
                    )
                    r3 = r[:, :]
                    r3 = bass.AP(
                        r3.tensor, r3.offset,
                        [r3.ap[0], [BS, 4], [1, BS]],
                    )
                    nc.vector.tensor_tensor(rZ[:], r3, zb_b, ALU.mult)

                    # chain: kneg = G(-2a).T @ rZ (4 accumulating mms)
                    kneg = ps_kneg.tile([H, BS], f32, tag="kneg")
                    gn = C("gneg")
                    for c in range(4):
                        nc.tensor.matmul(
                            kneg[:],
                            gn[:, (av * 4 + c) * H : (av * 4 + c + 1) * H],
                            rZ[:, c, :],
                            start=(c == 0), stop=(c == 3),
                        )

                    # bookkeeping (off chain) + next-stage input (chain)
                    if j == 0:
                        nc.vector.affine_then_add(A[:], korr[:], y[:], cj, 0.0)
                    else:
                        nc.vector.affine_then_add(A[:], korr[:], A[:], cj, 0.0)
                    nc.vector.affine_then_add(A[:], kneg[:], A[:], cj, 0.0)

                    if j < 3:
                        yk = w16.tile([H, BS], f32, tag="yk")
                        nc.vector.tensor_tensor(yk[:], y[:], korr[:], ALU.add)
                        ynext = ycpool.tile([H, BS], f32, tag="ycur")
                        nc.vector.tensor_tensor(ynext[:], yk[:], kneg[:], ALU.add)
                        ycur = ynext
                    else:
                        # A now holds y + sum_j u_j k_j = y_{t+1}
                        nc.vector.tensor_copy(y[:], A[:])

            # ---- readout: sigmoid(lw @ y + lb) ----
            pr = ps_korr.tile([1, BS], f32, tag="korr")
            nc.tensor.matmul(pr[:], C("lwT"), y[:], start=True, stop=True)
            er = w16.tile([1, BS], f32, tag="er")
            nc.scalar.activation(er[:], pr[:], AF.Exp, bias=C("lbneg"), scale=-1.0)
            dr = w16.tile([1, BS], f32, tag="dr")
            nc.vector.tensor_scalar_add(dr[:], er[:], 1.0)
            rr = w16.tile([1, BS], f32, tag="rr")
            nc.vector.reciprocal(rr[:], dr[:])
            nc.sync.dma_start(out_d[:], rr[:])

    nc.compile()
    return nc


def prep_inputs(ts, coeff_d, coeff_c, coeff_b, coeff_a,
                iw0, ib0, iw1, ib1, iw2, ib2,
                fw0, fb0, fw1, fb1, fw2, fb2, lw, lb, nsteps=NSTEPS_FULL):
    """Build per-core input maps (host-side numpy prep)."""
    f = np.float32
    cd = np.asarray(coeff_d, f)[:, :nsteps, :]
    cc = np.asarray(coeff_c, f)[:, :nsteps, :]
    cb = np.asarray(coeff_b, f)[:, :nsteps, :]
    ca = np.asarray(coeff_a, f)

    dX1 = cb
    dX23 = 0.75 * cd + cc + cb
    dX4 = 3.0 * cd + 2.0 * cc + cb

    fw2 = np.asarray(fw2, f)
    fb2 = np.asarray(fb2, f)

    def fill(wc, name, arr):
        p, o, fl = _L[name]
        assert arr.shape == (p, fl), (name, arr.shape, (p, fl))
        wc[0:p, o : o + fl] = arr

    wc0 = np.zeros((128, WCONST_F), f)
    fill(wc0, "fw0p", np.ascontiguousarray(np.asarray(fw0, f).T))
    fill(wc0, "fw1p", np.ascontiguousarray(np.asarray(fw1, f).T))
    fw2p = np.zeros((W, 512), f)
    b3l = np.zeros((4, 128), f)
    for c in range(4):
        for p in range(128):
            hd = _hd_orig(c, p)
            fw2p[:, c * 128 + p] = fw2[hd, :]
            b3l[c, p] = fb2[hd]
    fill(wc0, "fw2p", fw2p)
    fill(wc0, "b3l", b3l)
    b3r = np.zeros((4, 4 * BS), f)
    for c in range(4):
        b3r[c, c * BS : (c + 1) * BS] = 1.0
    fill(wc0, "b3r", b3r)
    gneg = np.zeros((128, 3 * 4 * H), f)
    for ai, aval in enumerate(_AVALS):
        for c in range(4):
            for p in range(128):
                h = 16 * c + (p % 16)
                gneg[p, (ai * 4 + c) * H + h] = -2.0 * aval
    fill(wc0, "gneg", gneg)
    onesa = np.zeros((D, 3 * H), f)
    for ai, aval in enumerate(_AVALS):
        onesa[:, ai * H : (ai + 1) * H] = aval
    fill(wc0, "onesa", onesa)
    ebc = np.zeros((D, 128), f)
    for p in range(128):
        ebc[p // 16, p] = 1.0
    fill(wc0, "ebc", ebc)
    fill(wc0, "iw0p", np.ascontiguousarray(np.asarray(iw0, f).T))
    fill(wc0, "iw1p", np.ascontiguousarray(np.asarray(iw1, f).T))
    fill(wc0, "iw2p", np.ascontiguousarray(np.asarray(iw2, f).T))
    fill(wc0, "lwT", np.ascontiguousarray(np.asarray(lw, f).reshape(1, H).T))
    fill(wc0, "ib0", np.asarray(ib0, f)[:, None])
    fill(wc0, "ib1", np.asarray(ib1, f)[:, None])
    fill(wc0, "ib2", np.asarray(ib2, f)[:, None])
    fill(wc0, "fb0", np.asarray(fb0, f)[:, None])
    fill(wc0, "fb1", np.asarray(fb1, f)[:, None])
    fill(wc0, "lbneg", -np.asarray(lb, f).reshape(1, 1))

    in_maps = []
    for i in range(NCORES):
        sl = slice(i * BS, (i + 1) * BS)
        wc = wc0.copy()
        fill(wc, "x0T", np.ascontiguousarray(ca[sl, 0, :].T))
        m = {"wconst": wc}
        for name, arr in (("dxt0", dX1), ("dxt1", dX23), ("dxt2", dX4)):
            m[name] = np.ascontiguousarray(
                arr[sl].transpose(2, 1, 0).reshape(D, -1)
            )
        in_maps.append(m)
    return in_maps


_CACHE = {}


def _get_nc(nsteps):
    if nsteps not in _CACHE:
        _CACHE[nsteps] = build_bass(nsteps)
    return _CACHE[nsteps]


def kernel(**inputs):
    from concourse.bass_utils import run_bass_kernel_spmd

    nsteps = NSTEPS_FULL
    in_maps = prep_inputs(nsteps=nsteps, **inputs)
    nc = _get_nc(nsteps)
    res = run_bass_kernel_spmd(nc, in_maps, list(range(NCORES)))
    outs = [res.results[i]["out"].reshape(BS) for i in range(NCORES)]
    return np.concatenate(outs, axis=0).astype(np.float32)



# revision 6
# speedup vs baseline: 1.3585x; 1.3585x over previous
"""Neural CDE Trainium2 kernel, v2.

Strategy: pure data parallelism over batch B=128 -> 8 cores x 16 rows.
Per core, the T-1=1023-step RK4 scan is a fully unrolled sequential
chain. Layout: activations [feature_on_partition, batch_on_free].

v2 redesign vs baseline (3x shorter critical chain per RK4 stage):

1. ONE activation table. The stock insert_act_table_loads pass resolves
   Exp -> exp_and_others and Ln -> natural_log, inserting a 1283ns
   LoadActFuncSet before nearly every activation (~10us/step). A Bacc
   subclass re-runs the pass with {Exp,Ln,Relu,Identity,Copy} stripped
   from every table except natural_log_exp_and_others, so the loop runs
   on a single resident table (one hoisted load).

2. tanh stays on ACT: tanh(v) = 1 - 2*exp(-softplus(2v)) via
   r = Exp(-Ln(1 + Exp(2v))), three back-to-back ACT ops (no table
   switch, no DVE excursion). k = S - 2*G^T(r.*Z) as in the baseline.

3. The state never leaves PSUM/matmul-land. Kernel state is
   fy_t = fw0 @ y_t (the layer-1 preactivation). Since ycur_{j+1} =
   y + a_j k_j and layer 1 is linear, every stage input preactivation
   is accumulated directly in PSUM:
     p1[j+1] = I128@fy + a_j*rowsum(fw0) (x) S  +  sum_c PM_c @ rZ_j
   where PM_c[p,w] = -2a * fw0[w, 16c+p%16] folds fw0 @ G^T(-2a) into
   one precomputed stationary. The step update fy_{t+1} accumulates the
   same way with u_j weights (PA_c) + a host-precomputed korr row KS_t.
   Seeds and korr matmuls are issued off the critical chain; only the
   4 PM/PA matmuls after each rZ are on it. y_T is recovered at readout
   with a single pinv(fw0) matmul (fw0 is 128x64, cond ~ 5.8).

   Per-stage chain: ACT{Exp,Ln} -> PE{p2} -> ACT{Exp,Ln} -> PE{p3 x4}
   -> ACT{Exp,Ln,Exp} -> DVE{rZ} -> PE{PM x4} -> next stage. 7 sem hops.

4. Z (dX replicated over 16 partitions/d) and the korr rows S are
   precomputed on host and streamed blockwise via double-buffered DMA
   (~25MB/core total, trivially overlapped).
"""

import numpy as np

B, T, D, H, W = 128, 1024, 8, 64, 128
NCORES = 8
BS = B // NCORES          # 16 batch rows per core
NSTEPS_FULL = T - 1       # 1023
TBLK = 96                 # steps per DMA block

_AJ = (0.5, 0.5, 1.0)          # stage input scale a_j for j=0,1,2
_UJ = (1.0 / 6.0, 1.0 / 3.0, 1.0 / 3.0, 1.0 / 6.0)  # y' weights
_SROW = (0, 1, 1, 2)           # dX variant per stage

# wconst free-dim layout: name -> (partitions, free_offset, free_len)
_L = {}
_off = 0
for _name, _p, _f in [
    ("fw1p", W, W), ("fw2p", W, 512),
    ("pm05", 128, 512), ("pm10", 128, 512),
    ("pa16", 128, 512), ("pa13", 128, 512),
    ("i128", 128, 128), ("b3l", 4, 128), ("b3r", 4, 64),
    ("fw0rs", 1, 128), ("fw0p", H, W),
    ("iw0p", D, W), ("iw1p", W, W), ("iw2p", W, H),
    ("x0T", D, BS), ("pinvT", 128, H), ("lwT", H, 1),
    ("fb0", W, 1), ("fb1", W, 1),
    ("ib0", W, 1), ("ib1", W, 1), ("ib2", H, 1),
    ("lbneg", 1, 1),
]:
    _L[_name] = (_p, _off, _f)
    _off += _f
WCONST_F = _off


def _nblk(nsteps):
    return (nsteps + TBLK - 1) // TBLK


def build_bass(nsteps):
    import concourse.bass as bass
    import concourse.bacc as bacc
    import concourse.mybir as mybir
    from concourse import tile
    from concourse.hw_specs import get_activation_tables
    import bass_rust as _bass_rust

    f32 = mybir.dt.float32
    AF = mybir.ActivationFunctionType
    ALU = mybir.AluOpType

    LOOP_FUNCS = {AF.Exp, AF.Ln, AF.Relu, AF.Identity, AF.Copy}
    ONE_TABLE = "natural_log_exp_and_others"

    class BaccOneTable(bacc.Bacc):
        """Bacc whose act-table pass may only satisfy the loop's
        activation functions from ONE table, so the fixpoint hoists a
        single LoadActFuncSet instead of thrashing tables per-op.
        Table ids stay canonical (same list order/names)."""

        def insert_act_table_loads(self):
            has_activation = any(
                isinstance(i, mybir.InstActivation)
                for b in self.main_func.blocks
                for i in b.instructions
            )
            if not has_activation:
                return
            tables = []
            for name, fns in get_activation_tables(self.m.arch).items():
                if name != ONE_TABLE:
                    fns = fns - LOOP_FUNCS
                tables.append((name, fns))
            _bass_rust.insert_act_table_loads(self, tables)

    nc = BaccOneTable(None)

    nblk = _nblk(nsteps)
    wc_d = nc.declare_dram_parameter("wconst", [128, WCONST_F], f32, isOutput=False)
    z_d = nc.declare_dram_parameter("zdat", [128, nblk * TBLK * 48], f32, isOutput=False)
    s_d = nc.declare_dram_parameter("sdat", [1, nblk * TBLK * 64], f32, isOutput=False)
    out_d = nc.declare_dram_parameter("out", [1, BS], f32, isOutput=True)

    with tile.TileContext(nc) as tc:
        with (
            tc.tile_pool(name="const", bufs=1) as cpool,
            tc.tile_pool(name="zblk", bufs=2) as zpool,
            tc.tile_pool(name="sblk", bufs=2) as spool,
            tc.tile_pool(name="fysb", bufs=2) as fypool,
            tc.tile_pool(name="work", bufs=2) as wpool,
            tc.tile_pool(name="rwork", bufs=2) as rpool,
            tc.tile_pool(name="rz", bufs=2) as rzpool,
            tc.tile_pool(name="ps_fy", bufs=2, space="PSUM") as ps_fy,
            tc.tile_pool(name="ps_r1", bufs=1, space="PSUM") as ps_r1,
            tc.tile_pool(name="ps_r2", bufs=1, space="PSUM") as ps_r2,
            tc.tile_pool(name="ps_r3", bufs=1, space="PSUM") as ps_r3,
            tc.tile_pool(name="ps_p2", bufs=1, space="PSUM") as ps_p2,
            tc.tile_pool(name="ps_p3", bufs=1, space="PSUM") as ps_p3,
            tc.tile_pool(name="ps_sc", bufs=1, space="PSUM") as ps_sc,
        ):
            wc = cpool.tile([128, WCONST_F], f32, tag="wconst")
            nc.sync.dma_start(wc[:], wc_d[:])

            zt = {}
            st = {}

            def load_blk(b):
                if b >= nblk:
                    return
                zt[b] = zpool.tile([128, TBLK * 48], f32, tag="z", name="ztile")
                nc.sync.dma_start(zt[b][:], z_d[:, b * TBLK * 48 : (b + 1) * TBLK * 48])
                st[b] = spool.tile([1, TBLK * 64], f32, tag="s", name="stile")
                nc.sync.dma_start(st[b][:], s_d[:, b * TBLK * 64 : (b + 1) * TBLK * 64])

            load_blk(0)
            load_blk(1)

            def C(name):
                p, o, f = _L[name]
                return wc[0:p, o : o + f]

            # Warm non-PE engines' vector clocks on the first DMAs so hot
            # ops don't carry a DMA wait alongside an engine wait.
            warm = wpool.tile([1, 4], f32, tag="warm")
            nc.scalar.activation(warm[0:1, 0:1], wc[0:1, 0:1], AF.Copy)
            nc.vector.tensor_copy(warm[0:1, 1:2], wc[0:1, 0:1])
            nc.vector.tensor_copy(warm[0:1, 2:3], zt[0][0:1, 0:1])
            nc.vector.tensor_copy(warm[0:1, 3:4], st[0][0:1, 0:1])

            # ---- init MLP: y0 = relu-MLP(x0); FY_0 = fw0 @ y0 ----
            pi1 = ps_p3.tile([W, BS], f32, tag="p3")
            nc.tensor.matmul(pi1[:], C("iw0p"), C("x0T"), start=True, stop=True)
            h1 = wpool.tile([W, BS], f32, tag="h")
            nc.scalar.activation(h1[:], pi1[:], AF.Relu, bias=C("ib0"))
            pi2 = ps_p3.tile([W, BS], f32, tag="p3")
            nc.tensor.matmul(pi2[:], C("iw1p"), h1[:], start=True, stop=True)
            h2 = wpool.tile([W, BS], f32, tag="h")
            nc.scalar.activation(h2[:], pi2[:], AF.Relu, bias=C("ib1"))
            pk = ps_p2.tile([H, BS], f32, tag="p2")
            nc.tensor.matmul(pk[:], C("iw2p"), h2[:], start=True, stop=True)
            y0 = wpool.tile([H, BS], f32, tag="h")
            nc.scalar.activation(y0[:], pk[:], AF.Identity, bias=C("ib2"))

            fyb = ps_fy.tile([128, BS], f32, tag="fy")
            nc.tensor.matmul(fyb[:], C("fw0p"), y0[:], start=True, stop=True)

            # ---- the scan ----
            for t in range(nsteps):
                b = t // TBLK
                toff = (t - b * TBLK)
                if toff == 0 and b + 1 < nblk:
                    load_blk(b + 1)
                    nc.vector.tensor_copy(warm[0:1, 2:3], zt[b + 1][0:1, 0:1])
                zcur = zt[b]
                scur = st[b]
                zoff = toff * 48
                soff = toff * 64

                # fy_t -> SBUF (off-chain; feeds all of step t's seeds)
                fy_sb = fypool.tile([128, BS], f32, tag="fysb")
                nc.vector.tensor_copy(fy_sb[:], fyb[:])

                # FY_{t+1} seed + korr row (off-chain)
                fyb_next = ps_fy.tile([128, BS], f32, tag="fy")
                nc.tensor.matmul(fyb_next[:], C("i128"), fy_sb[:], start=True, stop=False)
                nc.tensor.matmul(
                    fyb_next[:], C("fw0rs"), scur[0:1, soff : soff + BS],
                    start=False, stop=False,
                )
                # stage-input regions 1..3 seeds + korr rows (off-chain)
                rgs = []
                for j, pool in enumerate((ps_r1, ps_r2, ps_r3)):
                    rg = pool.tile([128, BS], f32, tag="rg", name="rgtile")
                    rgs.append(rg)
                    nc.tensor.matmul(rg[:], C("i128"), fy_sb[:], start=True, stop=False)
                    so = soff + (j + 1) * BS
                    nc.tensor.matmul(
                        rg[:], C("fw0rs"), scur[0:1, so : so + BS],
                        start=False, stop=False,
                    )

                for j in range(4):
                    pin = fyb[:] if j == 0 else rgs[j - 1][:]

                    # p3 bank bias seed (off-chain: only WAR on prev stage)
                    p3b = ps_p3.tile([128, 4 * BS], f32, tag="p3")
                    nc.tensor.matmul(p3b[:], C("b3l"), C("b3r"), start=True, stop=False)

                    sc = ps_sc.tile([128, 32], f32, tag="sc")

                    # layer 1: softplus(p1) ; p1 = pin + fb0
                    nc.scalar.activation(sc[:, 0:16], pin, AF.Exp, bias=C("fb0"))
                    s1 = wpool.tile([W, BS], f32, tag="h")
                    nc.scalar.activation(s1[:], sc[:, 0:16], AF.Ln, bias=1.0)

                    # layer 2
                    p2b = ps_p2.tile([W, BS], f32, tag="p2")
                    nc.tensor.matmul(p2b[:], C("fw1p"), s1[:], start=True, stop=True)
                    nc.scalar.activation(sc[:, 16:32], p2b[:], AF.Exp, bias=C("fb1"))
                    s2 = wpool.tile([W, BS], f32, tag="h")
                    nc.scalar.activation(s2[:], sc[:, 16:32], AF.Ln, bias=1.0)

                    # layer 3 (4 chunks into the bias-seeded bank)
                    fw2p = C("fw2p")
                    for c in range(4):
                        nc.tensor.matmul(
                            p3b[:, c * BS : (c + 1) * BS],
                            fw2p[:, c * 128 : (c + 1) * 128],
                            s2[:],
                            start=False, stop=(c == 3),
                        )

                    # r = 1/(1+e^{2v}): Exp on ACT, clamp+1 and recip on DVE
                    texp = rpool.tile([128, 4 * BS], f32, tag="texp")
                    nc.scalar.activation(texp[:], p3b[:], AF.Exp, scale=2.0)
                    den = rpool.tile([128, 4 * BS], f32, tag="den")
                    nc.vector.tensor_scalar(
                        den[:], texp[:], 1.0e30, 1.0, ALU.min, ALU.add
                    )
                    r = rpool.tile([128, 4 * BS], f32, tag="r")
                    nc.vector.reciprocal_approx_fast(r[:], den[:])

                    # rZ = r .* Z_{s(j)}  (Z broadcast along the 4 chunks)
                    s_ = _SROW[j]
                    zsl = zcur[:, zoff + s_ * BS : zoff + (s_ + 1) * BS]
                    zb = bass.AP(
                        zsl.tensor, zsl.offset, [zsl.ap[0], [0, 4], zsl.ap[1]]
                    )
                    r3 = bass.AP(
                        r[:, :].tensor, r[:, :].offset,
                        [r[:, :].ap[0], [BS, 4], [1, BS]],
                    )
                    rz = rzpool.tile([128, 4, BS], f32, tag="rz")
                    nc.vector.tensor_tensor(rz[:], r3, zb, ALU.mult)

                    # chain: stage-input for j+1 (PM); off-chain: fy (PA)
                    if j < 3:
                        pm = C("pm05") if _AJ[j] == 0.5 else C("pm10")
                        for c in range(4):
                            nc.tensor.matmul(
                                rgs[j][:], pm[:, c * 128 : (c + 1) * 128], rz[:, c, :],
                                start=False, stop=(c == 3),
                            )
                    pa = C("pa16") if j in (0, 3) else C("pa13")
                    for c in range(4):
                        nc.tensor.matmul(
                            fyb_next[:], pa[:, c * 128 : (c + 1) * 128], rz[:, c, :],
                            start=False, stop=(j == 3 and c == 3),
                        )

                fyb = fyb_next

            # ---- readout: sigmoid(lw @ pinv(fw0) @ fy_T + lb) ----
            fyT = fypool.tile([128, BS], f32, tag="fysb")
            nc.vector.tensor_copy(fyT[:], fyb[:])
            ytp = ps_p2.tile([H, BS], f32, tag="p2")
            nc.tensor.matmul(ytp[:], C("pinvT"), fyT[:], start=True, stop=True)
            yt = wpool.tile([H, BS], f32, tag="h")
            nc.scalar.activation(yt[:], ytp[:], AF.Copy)
            pr = ps_p3.tile([1, BS], f32, tag="p3")
            nc.tensor.matmul(pr[:], C("lwT"), yt[:], start=True, stop=True)
            er = wpool.tile([1, BS], f32, tag="warm2")
            nc.scalar.activation(er[:], pr[:], AF.Exp, bias=C("lbneg"), scale=-1.0)
            dr = wpool.tile([1, BS], f32, tag="warm2")
            nc.vector.tensor_scalar_add(dr[:], er[:], 1.0)
            rr = wpool.tile([1, BS], f32, tag="warm2")
            nc.vector.reciprocal(rr[:], dr[:])
            nc.sync.dma_start(out_d[:], rr[:])

    nc.compile()
    return nc


def prep_inputs(ts, coeff_d, coeff_c, coeff_b, coeff_a,
                iw0, ib0, iw1, ib1, iw2, ib2,
                fw0, fb0, fw1, fb1, fw2, fb2, lw, lb, nsteps=NSTEPS_FULL):
    """Build per-core input maps (host-side numpy prep)."""
    f = np.float32
    cd = np.asarray(coeff_d, f)[:, :nsteps, :]
    cc = np.asarray(coeff_c, f)[:, :nsteps, :]
    cb = np.asarray(coeff_b, f)[:, :nsteps, :]
    ca = np.asarray(coeff_a, f)

    # dX variants per RK4 stage (h == 1): s=0 @ t, s=1 @ t+1/2, s=2 @ t+1
    dX = [cb, 0.75 * cd + cc + cb, 3.0 * cd + 2.0 * cc + cb]  # [B,nsteps,D]

    fw0 = np.asarray(fw0, f)
    fw2 = np.asarray(fw2, f)
    fb2 = np.asarray(fb2, f)

    def fill(wcv, name, arr):
        p, o, fl = _L[name]
        assert arr.shape == (p, fl), (name, arr.shape, (p, fl))
        wcv[0:p, o : o + fl] = arr

    p_ar = np.arange(128)
    hmap = 16 * (np.arange(4)[:, None] // 1) + 0  # placeholder

    wc0 = np.zeros((128, WCONST_F), f)
    fill(wc0, "fw1p", np.ascontiguousarray(np.asarray(fw1, f).T))

    # fw2p[w, c*128 + p] = fw2[hd(c,p), w],  hd = (16c + p%16)*D + p//16
    fw2p = np.zeros((W, 512), f)
    b3l = np.zeros((4, 128), f)
    for c in range(4):
        h = 16 * c + (p_ar % 16)
        d = p_ar // 16
        hd = h * D + d
        fw2p[:, c * 128 + p_ar] = fw2[hd, :].T
        b3l[c, p_ar] = fb2[hd]
    fill(wc0, "fw2p", fw2p)
    fill(wc0, "b3l", b3l)
    b3r = np.zeros((4, 64), f)
    for c in range(4):
        b3r[c, c * BS : (c + 1) * BS] = 1.0
    fill(wc0, "b3r", b3r)

    # PM/PA[p, c*128 + w] = scal * fw0[w, 16c + p%16]
    def pmat(scal):
        m = np.zeros((128, 512), f)
        for c in range(4):
            h = 16 * c + (p_ar % 16)
            m[p_ar[:, None], c * 128 + np.arange(W)[None, :]] = scal * fw0[:, h].T
        return m

    fill(wc0, "pm05", pmat(-2.0 * 0.5))
    fill(wc0, "pm10", pmat(-2.0 * 1.0))
    fill(wc0, "pa16", pmat(-2.0 / 6.0))
    fill(wc0, "pa13", pmat(-2.0 / 3.0))
    fill(wc0, "i128", np.eye(128, dtype=f))
    fill(wc0, "fw0rs", fw0.sum(axis=1)[None, :])
    fill(wc0, "fw0p", np.ascontiguousarray(fw0.T))
    fill(wc0, "iw0p", np.ascontiguousarray(np.asarray(iw0, f).T))
    fill(wc0, "iw1p", np.ascontiguousarray(np.asarray(iw1, f).T))
    fill(wc0, "iw2p", np.ascontiguousarray(np.asarray(iw2, f).T))
    pinv = np.linalg.pinv(fw0.astype(np.float64)).astype(f)  # [H, 128]
    fill(wc0, "pinvT", np.ascontiguousarray(pinv.T))
    fill(wc0, "lwT", np.ascontiguousarray(np.asarray(lw, f).reshape(1, H).T))
    fill(wc0, "fb0", np.asarray(fb0, f)[:, None])
    fill(wc0, "fb1", np.asarray(fb1, f)[:, None])
    fill(wc0, "ib0", np.asarray(ib0, f)[:, None])
    fill(wc0, "ib1", np.asarray(ib1, f)[:, None])
    fill(wc0, "ib2", np.asarray(ib2, f)[:, None])
    fill(wc0, "lbneg", -np.asarray(lb, f).reshape(1, 1))

    nblk = _nblk(nsteps)
    npad = nblk * TBLK

    # korr rows: S_s[b,t] = sum_d dX_s ; sdat row r at step t:
    #   r=0: KS = sum_j u_j S_{s(j)} ; r=1..3: a_j * S_{s(j)}
    S = [d_.sum(axis=2) for d_ in dX]  # [B, nsteps]
    KS = (1.0 / 6.0) * S[0] + (2.0 / 3.0) * S[1] + (1.0 / 6.0) * S[2]
    rows = [KS, 0.5 * S[0], 0.5 * S[1], 1.0 * S[1]]

    in_maps = []
    for i in range(NCORES):
        sl = slice(i * BS, (i + 1) * BS)
        wcv = wc0.copy()
        fill(wcv, "x0T", np.ascontiguousarray(ca[sl, 0, :].T))

        # zdat[p, t*48 + s*16 + b] = dX_s[b, t, p//16]
        z = np.zeros((128, npad, 3, BS), f)
        for s_ in range(3):
            z[:, :nsteps, s_, :] = np.repeat(
                dX[s_][sl].transpose(2, 1, 0), 16, axis=0
            )
        z = np.ascontiguousarray(z.reshape(128, npad * 48))

        sd = np.zeros((npad, 4, BS), f)
        for r_ in range(4):
            sd[:nsteps, r_, :] = rows[r_][sl].T
        sd = np.ascontiguousarray(sd.reshape(1, npad * 64))

        in_maps.append({"wconst": wcv, "zdat": z, "sdat": sd})
    return in_maps


_CACHE = {}


def _get_nc(nsteps):
    if nsteps not in _CACHE:
        _CACHE[nsteps] = build_bass(nsteps)
    return _CACHE[nsteps]


def kernel(**inputs):
    from concourse.bass_utils import run_bass_kernel_spmd

    nsteps = NSTEPS_FULL
    in_maps = prep_inputs(nsteps=nsteps, **inputs)
    nc = _get_nc(nsteps)
    res = run_bass_kernel_spmd(nc, in_maps, list(range(NCORES)))
    outs = [res.results[i]["out"].reshape(BS) for i in range(NCORES)]
    return np.concatenate(outs, axis=0).astype(np.float32)
